# revision 1
# baseline (speedup 1.0000x reference)
"""Trainium2 Bass kernel for CustomFullyConnectedLayerGoogleTopK.

Math (from the reference, with IN_F == OUT_F == TOTAL_PERM == DIAG_LEN == 4096):
    a_topk = clip(K * softmax(alpha), 0, 1)                    # K = 3687
    Vs     = V * a_topk[:, None]                               # [4096, 4096]
    W[r,c] = Vs[(r - c) % 4096, c]   (scatter has no collisions)
    out    = x @ W.T                                           # [8192, 4096]

Device strategy: data-parallel over batch (8 cores x 1024 rows). The weight
W.T[c, r] = VsT[c, (r - c) % 4096] where VsT = Vs.T. Storing the doubled
array W2 = concat(VsT, VsT, axis=1) [4096, 8192] makes every [128, ntile]
tile of W.T a single affine access pattern: element (p, j) of the tile for
(k, n) lives at W2 linear offset (4096 + n*ntile) + p*8191 + k*128*8191 + j.
So the whole matmul streams with plain DMAs - no gather, no transpose.

Each core: out_slice[1024, 4096] = xT_slice.T @ W.T via PE matmuls,
lhsT = xT tile [128c, 128b] (stationary), rhs = W.T tile [128c, ntile r].

Measured (core 0, NTFF profile): ~458 us HW exec in bf16; the matmul
stream itself runs gap-free at the PE issue-rate floor (442 us for 2048
N=512 matmuls), the rest is the framework start barrier + end drain.
Absmax-relative error 1.9e-3.
GTOPK_MODE=fp32r gives 1.2e-4 rel err at ~566 us if tighter accuracy is
ever needed.
"""

import os

import numpy as np
import ml_dtypes

B = 8192  # batch
F = 4096  # in_features == out_features == total_perm == diag_len
NCORES = 8
BS = B // NCORES  # batch rows per core
KTOPK = 3687  # ceil((1 - 0.1) * F * F / F)

# "bf16" (full-rate PE, ~1.6e-3 rel err) or "fp32r" (fp32 storage,
# reduced-precision multiply).
_MODE = os.environ.get("GTOPK_MODE", "bf16")

_NC_CACHE = {}
_LAST_RESULTS = None  # stashed BassKernelResults for test harness introspection


def _build_nc(mode, f=F, bs=BS):
    import concourse.bass as bass
    import concourse.tile as tile
    from concourse import bacc, mybir

    if mode == "bf16":
        in_dt = mybir.dt.bfloat16
        n_tile = 512
    elif mode == "fp32r":
        in_dt = mybir.dt.float32r
        n_tile = 256
    else:
        raise ValueError(mode)

    k_tiles = f // 128
    m_tiles = bs // 128
    n_tiles = f // n_tile
    w2w = 2 * f  # doubled width

    nc = bacc.Bacc(None, target_bir_lowering=False, debug=False)
    xt = nc.dram_tensor("xt", [f, bs], in_dt, kind="ExternalInput")
    w2 = nc.dram_tensor("w2", [f, w2w], in_dt, kind="ExternalInput")
    out = nc.dram_tensor("out", [bs, f], mybir.dt.float32, kind="ExternalOutput")

    def xt_src(k):  # [128, bs] tile k of x.T slice
        return bass.AP(xt, k * 128 * bs, [[bs, 128], [1, bs]])

    def wt_src(n, k):  # staircase [128, n_tile] tile of W.T
        return bass.AP(
            w2, f + n * n_tile + k * 128 * (w2w - 1), [[w2w - 1, 128], [1, n_tile]]
        )

    with tile.TileContext(nc) as tc:
        with (
            tc.tile_pool(name="xpool", bufs=k_tiles + 1) as xpool,
            tc.tile_pool(name="wpool", bufs=3 * (k_tiles // 2)) as wpool,
            tc.tile_pool(name="opool", bufs=6) as opool,
            tc.tile_pool(name="ppool", bufs=8, space="PSUM") as ppool,
        ):
            # HAM warmup: N=128 matmuls on scratch fill the PE-idle window
            # between the framework start barrier and first data arrival, so
            # real matmuls start at the warm 2.4 GHz clock. 32 covers the
            # worst-case window (preamble end varies +-2us run to run; too
            # few dummies = PE idles then runs cold, measured +1us).
            warm = xpool.tile([128, 128], in_dt, name="warm", tag="warm", bufs=1)
            nc.vector.memset(warm[:], 0)
            ps_w = ppool.tile([128, n_tile], mybir.dt.float32, name="ps_w", tag="ps")
            for _ in range(32):
                nc.tensor.matmul(
                    ps_w[:, :128], warm[:], warm[:],
                    start=True, stop=True, skip_group_check=True,
                )

            # x.T slice cached in SBUF as separate tiles so the scheduler can
            # start matmuls as soon as individual tiles land. xt[0] is split
            # so the very first (xt, wt) pair is only 192 KB. Interleave x/w
            # DMAs for n=0 so pairs arrive together.
            xts = []
            wt0 = []
            split0 = bs > 256
            if split0:
                x0a = xpool.tile([128, 256], in_dt, name="xt0a", tag="xt")
                nc.sync.dma_start(
                    out=x0a[:], in_=bass.AP(xt, 0, [[bs, 128], [1, 256]])
                )
                w0 = wpool.tile([128, n_tile], in_dt, name="wt0_0", tag="wt")
                nc.sync.dma_start(out=w0[:], in_=wt_src(0, 0))
                x0b = xpool.tile([128, bs - 256], in_dt, name="xt0b", tag="xt")
                nc.sync.dma_start(
                    out=x0b[:], in_=bass.AP(xt, 256, [[bs, 128], [1, bs - 256]])
                )
                xts.append((x0a, x0b))
                wt0.append(w0)
            else:
                x0 = xpool.tile([128, bs], in_dt, name="xt0", tag="xt")
                nc.sync.dma_start(out=x0[:], in_=xt_src(0))
                w0 = wpool.tile([128, n_tile], in_dt, name="wt0_0", tag="wt")
                nc.sync.dma_start(out=w0[:], in_=wt_src(0, 0))
                xts.append(x0)
                wt0.append(w0)
            for k in range(1, k_tiles):
                xk = xpool.tile([128, bs], in_dt, name=f"xt{k}", tag="xt")
                nc.sync.dma_start(out=xk[:], in_=xt_src(k))
                wk = wpool.tile([128, n_tile], in_dt, name=f"wt0_{k}", tag="wt")
                nc.sync.dma_start(out=wk[:], in_=wt_src(0, k))
                xts.append(xk)
                wt0.append(wk)

            def xsl(k, m):  # lhsT block [128, 128] for (k-tile, m-tile)
                if k == 0 and split0:
                    a, b = xts[0]
                    if m < 2:
                        return a[:, m * 128 : (m + 1) * 128]
                    return b[:, (m - 2) * 128 : (m - 1) * 128]
                return xts[k][:, m * 128 : (m + 1) * 128]

            wts = wt0
            for n in range(n_tiles):
                # prefetch next n's weight tiles (2 k-tiles per DMA: halves
                # the ~0.6us-per-DMA issue load on the sync sequencer)
                if n + 1 < n_tiles:
                    nxt = []
                    for k2 in range(k_tiles // 2):
                        wk = wpool.tile(
                            [128, 2, n_tile], in_dt, name=f"wt{n + 1}_{k2}", tag="wt"
                        )
                        nc.sync.dma_start(
                            out=wk[:],
                            in_=bass.AP(
                                w2,
                                f + (n + 1) * n_tile + k2 * 256 * (w2w - 1),
                                [[w2w - 1, 128], [128 * (w2w - 1), 2], [1, n_tile]],
                            ),
                        )
                        nxt.append(wk)
                def wsl(k):  # rhs [128, n_tile] for k-tile of current n
                    if n == 0:
                        return wts[k][:]
                    return wts[k // 2][:, k % 2, :]

                def evict(ps_ap, m, col0, width):
                    o_sb = opool.tile(
                        [128, width], mybir.dt.float32, name="o_sb", tag="o_sb"
                    )
                    nc.vector.tensor_copy(o_sb[:], ps_ap)
                    nc.scalar.dma_start(
                        out=bass.AP(
                            out, m * 128 * f + n * n_tile + col0, [[f, 128], [1, width]]
                        ),
                        in_=o_sb[:],
                    )

                if n == 0:
                    # Ramp phase: k-outer / m-inner over the first half of k
                    # so each arriving (xt[k], wt[k]) pair immediately feeds
                    # m_tiles matmuls (PE starts as soon as the first pair
                    # lands). Then finish per-m (k-inner) so the 8 psum banks
                    # complete staggered and evictions overlap compute.
                    k_half = min(3 * k_tiles // 4, k_tiles)
                    pss = [
                        ppool.tile([128, n_tile], mybir.dt.float32, name=f"ps{m}", tag="ps")
                        for m in range(m_tiles)
                    ]
                    for k in range(k_half):
                        for m in range(m_tiles):
                            nc.tensor.matmul(
                                pss[m][:],
                                xsl(k, m),
                                wsl(k),
                                start=(k == 0),
                                stop=False,
                                skip_group_check=True,
                            )
                    for m in range(m_tiles):
                        for k in range(k_half, k_tiles):
                            nc.tensor.matmul(
                                pss[m][:],
                                xsl(k, m),
                                wsl(k),
                                start=False,
                                stop=(k == k_tiles - 1),
                                skip_group_check=True,
                            )
                        evict(pss[m][:], m, 0, n_tile)
                else:
                    # m-outer / k-inner: staggered psum completion overlaps
                    # eviction + output DMA with compute. The very last group
                    # is split in half column-wise so the final eviction +
                    # output DMA (whose ~2us HBM write receipt is on the
                    # critical path) moves half as much data after the last
                    # matmul.
                    for m in range(m_tiles):
                        last = n == n_tiles - 1 and m == m_tiles - 1
                        if not last:
                            ps = ppool.tile(
                                [128, n_tile], mybir.dt.float32, name="ps", tag="ps"
                            )
                            for k in range(k_tiles):
                                nc.tensor.matmul(
                                    ps[:],
                                    xsl(k, m),
                                    wsl(k),
                                    start=(k == 0),
                                    stop=(k == k_tiles - 1),
                                )
                            evict(ps[:], m, 0, n_tile)
                        else:
                            half = n_tile // 2
                            for h in range(2):
                                ps = ppool.tile(
                                    [128, half], mybir.dt.float32, name="ps", tag="ps"
                                )
                                for k in range(k_tiles):
                                    nc.tensor.matmul(
                                        ps[:],
                                        xsl(k, m),
                                        wsl(k)[:, h * half : (h + 1) * half],
                                        start=(k == 0),
                                        stop=(k == k_tiles - 1),
                                    )
                                evict(ps[:], m, h * half, half)
                if n + 1 < n_tiles:
                    wts = nxt
    nc.compile()
    return nc


def _get_nc(mode):
    if mode not in _NC_CACHE:
        _NC_CACHE[mode] = _build_nc(mode)
    return _NC_CACHE[mode]


def _soft_topk_scale(alpha):
    a = alpha.astype(np.float64)
    e = np.exp(a - a.max())
    return np.clip(KTOPK * (e / e.sum()), 0.0, 1.0).astype(np.float32)


def kernel(x, V, alpha):
    global _LAST_RESULTS
    from concourse.bass_utils import run_bass_kernel_spmd

    x = np.asarray(x, dtype=np.float32)
    V = np.asarray(V, dtype=np.float32)
    alpha = np.asarray(alpha, dtype=np.float32)

    a_topk = _soft_topk_scale(alpha)
    VsT = np.ascontiguousarray((V * a_topk[:, None]).T)  # [c, p]
    W2 = np.concatenate([VsT, VsT], axis=1)  # [F, 2F]
    xT = np.ascontiguousarray(x.T)  # [F, B]

    mode = _MODE
    if mode == "bf16":
        W2 = W2.astype(ml_dtypes.bfloat16)
        xT = xT.astype(ml_dtypes.bfloat16)

    nc = _get_nc(mode)
    in_maps = [
        {"xt": np.ascontiguousarray(xT[:, i * BS : (i + 1) * BS]), "w2": W2}
        for i in range(NCORES)
    ]
    kwargs = {}
    if os.environ.get("GTOPK_TRACE"):
        try:
            import antenv.axon_hooks  # noqa: F401  (trace needs the hook)

            kwargs["trace"] = True
        except ImportError:
            pass
    res = run_bass_kernel_spmd(nc, in_maps, core_ids=list(range(NCORES)), **kwargs)
    _LAST_RESULTS = res
    return np.concatenate([r["out"] for r in res.results], axis=0)



# revision 2
# speedup vs baseline: 1.0963x; 1.0963x over previous
"""Trainium2 Bass kernel for CustomFullyConnectedLayerGoogleTopK.

Math (from the reference, with IN_F == OUT_F == TOTAL_PERM == DIAG_LEN == 4096):
    a_topk = clip(K * softmax(alpha), 0, 1)                    # K = 3687
    Vs     = V * a_topk[:, None]                               # [4096, 4096]
    W[r,c] = Vs[(r - c) % 4096, c]   (scatter has no collisions)
    out    = x @ W.T                                           # [8192, 4096]

Device strategy: data-parallel over batch (8 cores x 1024 rows). The weight
W.T[c, r] = VsT[c, (r - c) % 4096] where VsT = Vs.T. Storing the doubled
array W2 = concat(VsT, VsT, axis=1) [4096, 8192] makes every [128, ntile]
tile of W.T a single affine access pattern - the whole matmul streams with
plain DMAs, no gather, no transpose.

Each core: out_slice[1024, 4096] = xT_slice.T @ W.T via PE matmuls,
lhsT = xT tile (stationary), rhs = W.T tile [contract, ntile r].

Mixed-precision split-K ("mixed" mode, default): the bf16 MM stream is at
the PE issue-rate floor (~216 ns per N=512 matmul, 442 us/core), so the
only way below it is a cheaper dtype. fp8e4 with perf_mode=DoubleRow
contracts 256 rows per matmul (2 fp8 weights/cell) at ~1.5x bf16
throughput, but full-fp8 error (~2.7e-2 absmax-rel) exceeds the 2e-2
gate. Quantization error grows as sqrt(fraction of K in fp8), so the last
6 of 32 k-tiles (3 DoubleRow pairs) run in fp8: measured 1.3e-2 absmax /
1.7e-2 L2 rel err at full batch, and ~35 us faster. W values (~±2^-6) sit
in e4m3's subnormal range, so W carries a 2^13 scale (folded into the bf16
W too, removed exactly at eviction with a tensor_scalar_mul by 2^-13).

GTOPK_MODE=bf16 falls back to the pure-bf16 kernel (1.9e-3 rel err).
"""

import os

import numpy as np
import ml_dtypes

B = 8192  # batch
F = 4096  # in_features == out_features == total_perm == diag_len
NCORES = 8
BS = B // NCORES  # batch rows per core
KTOPK = 3687  # ceil((1 - 0.1) * F * F / F)

# fp8 k-pairs (2 k-tiles of 128 each) in mixed mode; 3 => 6/32 of K in fp8
KF8_PAIRS = int(os.environ.get("GTOPK_KF8", "3"))
WSCALE_LOG2 = 13  # W pre-scale so fp8 W values land in e4m3 normal range

_MODE = os.environ.get("GTOPK_MODE", "mixed")

_NC_CACHE = {}
_LAST_RESULTS = None  # stashed BassKernelResults for test harness introspection


def _build_nc(mode, f=F, bs=BS, n_tile=512, kf8_pairs=KF8_PAIRS):
    import concourse.bass as bass
    import concourse.tile as tile
    from concourse import bacc, mybir

    mixed = mode == "mixed"
    if not mixed:
        kf8_pairs = 0

    in_dt = mybir.dt.bfloat16
    f8_dt = mybir.dt.float8e4
    dr_mode = mybir.MatmulPerfMode.DoubleRow

    k_tiles = f // 128
    kbf = k_tiles - 2 * kf8_pairs  # bf16 k-tiles
    assert kbf >= 2 and kbf % 2 == 0
    m_tiles = bs // 128
    n_tiles = f // n_tile
    w2w = 2 * f  # doubled width
    evs = float(2.0**-WSCALE_LOG2)

    nc = bacc.Bacc(None, target_bir_lowering=False, debug=False)
    xt = nc.dram_tensor("xt", [kbf * 128, bs], in_dt, kind="ExternalInput")
    w2 = nc.dram_tensor("w2", [kbf * 128, w2w], in_dt, kind="ExternalInput")
    if kf8_pairs:
        xt8 = nc.dram_tensor("xt8", [kf8_pairs * 256, bs], f8_dt, kind="ExternalInput")
        w28 = nc.dram_tensor("w28", [kf8_pairs * 256, w2w], f8_dt, kind="ExternalInput")
    out = nc.dram_tensor("out", [bs, f], mybir.dt.float32, kind="ExternalOutput")

    def xt_src(k):  # [128, bs] tile k of x.T slice
        return bass.AP(xt, k * 128 * bs, [[bs, 128], [1, bs]])

    def wt_src(n, k):  # staircase [128, n_tile] tile of W.T
        return bass.AP(
            w2, f + n * n_tile + k * 128 * (w2w - 1), [[w2w - 1, 128], [1, n_tile]]
        )

    def x8_src(t):  # [128, 2, bs] fp8 x pair t (k-tiles kbf+2t, kbf+2t+1)
        return bass.AP(xt8, t * 256 * bs, [[bs, 128], [128 * bs, 2], [1, bs]])

    def w8_src(n, t):  # staircase [128, 2, n_tile] fp8 W.T pair
        return bass.AP(
            w28,
            f + n * n_tile - kbf * 128 + t * 256 * (w2w - 1),
            [[w2w - 1, 128], [128 * (w2w - 1), 2], [1, n_tile]],
        )

    with tile.TileContext(nc) as tc:
        with (
            tc.tile_pool(name="xpool", bufs=kbf + 1 + 2 * kf8_pairs) as xpool,
            tc.tile_pool(name="wpool", bufs=3 * (kbf // 2)) as wpool,
            tc.tile_pool(name="w8pool", bufs=max(1, 3 * kf8_pairs)) as w8pool,
            tc.tile_pool(name="opool", bufs=6) as opool,
            tc.tile_pool(name="ppool", bufs=8, space="PSUM") as ppool,
        ):
            # HAM warmup: N=128 matmuls on scratch fill the PE-idle window
            # between the framework start barrier and first data arrival, so
            # real matmuls start at the warm 2.4 GHz clock.
            warm = xpool.tile([128, 128], in_dt, name="warm", tag="warm", bufs=1)
            nc.vector.memset(warm[:], 0)
            ps_w = ppool.tile([128, n_tile], mybir.dt.float32, name="ps_w", tag="ps")
            for _ in range(32):
                nc.tensor.matmul(
                    ps_w[:, :128], warm[:], warm[:],
                    start=True, stop=True, skip_group_check=True,
                )

            # x.T slice cached in SBUF as separate tiles so the scheduler can
            # start matmuls as soon as individual tiles land. xt[0] is split
            # so the very first (xt, wt) pair is only 192 KB. Interleave x/w
            # DMAs for n=0 so pairs arrive together.
            xts = []
            wt0 = []
            split0 = bs > 256
            if split0:
                x0a = xpool.tile([128, 256], in_dt, name="xt0a", tag="xt")
                nc.sync.dma_start(
                    out=x0a[:], in_=bass.AP(xt, 0, [[bs, 128], [1, 256]])
                )
                w0 = wpool.tile([128, n_tile], in_dt, name="wt0_0", tag="wt")
                nc.sync.dma_start(out=w0[:], in_=wt_src(0, 0))
                x0b = xpool.tile([128, bs - 256], in_dt, name="xt0b", tag="xt")
                nc.sync.dma_start(
                    out=x0b[:], in_=bass.AP(xt, 256, [[bs, 128], [1, bs - 256]])
                )
                xts.append((x0a, x0b))
                wt0.append(w0)
            else:
                x0 = xpool.tile([128, bs], in_dt, name="xt0", tag="xt")
                nc.sync.dma_start(out=x0[:], in_=xt_src(0))
                w0 = wpool.tile([128, n_tile], in_dt, name="wt0_0", tag="wt")
                nc.sync.dma_start(out=w0[:], in_=wt_src(0, 0))
                xts.append(x0)
                wt0.append(w0)
            for k in range(1, kbf):
                xk = xpool.tile([128, bs], in_dt, name=f"xt{k}", tag="xt")
                nc.sync.dma_start(out=xk[:], in_=xt_src(k))
                wk = wpool.tile([128, n_tile], in_dt, name=f"wt0_{k}", tag="wt")
                nc.sync.dma_start(out=wk[:], in_=wt_src(0, k))
                xts.append(xk)
                wt0.append(wk)
            x8ts = []
            w8t0 = []
            for t in range(kf8_pairs):
                x8k = xpool.tile([128, 2, bs], f8_dt, name=f"x8t{t}", tag="xt")
                nc.sync.dma_start(out=x8k[:], in_=x8_src(t))
                w8k = w8pool.tile([128, 2, n_tile], f8_dt, name=f"w8t0_{t}", tag="w8")
                nc.sync.dma_start(out=w8k[:], in_=w8_src(0, t))
                x8ts.append(x8k)
                w8t0.append(w8k)

            def xsl(k, m):  # lhsT block [128, 128] for (k-tile, m-tile)
                if k == 0 and split0:
                    a, b = xts[0]
                    if m < 2:
                        return a[:, m * 128 : (m + 1) * 128]
                    return b[:, (m - 2) * 128 : (m - 1) * 128]
                return xts[k][:, m * 128 : (m + 1) * 128]

            def x8sl(t, m):  # DR lhsT [128, 2, 128]
                return x8ts[t][:, :, m * 128 : (m + 1) * 128]

            wts = wt0
            w8ts = w8t0
            for n in range(n_tiles):
                # prefetch next n's weight tiles (2 k-tiles per DMA: halves
                # the ~0.6us-per-DMA issue load on the sync sequencer)
                if n + 1 < n_tiles:
                    nxt = []
                    for k2 in range(kbf // 2):
                        wk = wpool.tile(
                            [128, 2, n_tile], in_dt, name=f"wt{n + 1}_{k2}", tag="wt"
                        )
                        nc.sync.dma_start(
                            out=wk[:],
                            in_=bass.AP(
                                w2,
                                f + (n + 1) * n_tile + k2 * 256 * (w2w - 1),
                                [[w2w - 1, 128], [128 * (w2w - 1), 2], [1, n_tile]],
                            ),
                        )
                        nxt.append(wk)
                    nxt8 = []
                    for t in range(kf8_pairs):
                        w8k = w8pool.tile(
                            [128, 2, n_tile], f8_dt, name=f"w8t{n + 1}_{t}", tag="w8"
                        )
                        nc.sync.dma_start(out=w8k[:], in_=w8_src(n + 1, t))
                        nxt8.append(w8k)

                def wsl(k):  # rhs [128, n_tile] for k-tile of current n
                    if n == 0:
                        return wts[k][:]
                    return wts[k // 2][:, k % 2, :]

                def dr_mms(ps_ap, m, col0, width, skip):
                    for t in range(kf8_pairs):
                        rhs = w8ts[t][:]
                        if width != n_tile:
                            rhs = w8ts[t][:, :, col0 : col0 + width]
                        nc.tensor.matmul(
                            ps_ap,
                            x8sl(t, m),
                            rhs,
                            start=False,
                            stop=(t == kf8_pairs - 1),
                            perf_mode=dr_mode,
                            skip_group_check=skip,
                        )

                def evict(ps_ap, m, col0, width):
                    o_sb = opool.tile(
                        [128, width], mybir.dt.float32, name="o_sb", tag="o_sb"
                    )
                    if mixed:
                        nc.vector.tensor_scalar_mul(o_sb[:], ps_ap, evs)
                    else:
                        nc.vector.tensor_copy(o_sb[:], ps_ap)
                    nc.scalar.dma_start(
                        out=bass.AP(
                            out, m * 128 * f + n * n_tile + col0, [[f, 128], [1, width]]
                        ),
                        in_=o_sb[:],
                    )

                if n == 0:
                    # Ramp phase: k-outer / m-inner over the first chunk of k
                    # so each arriving (xt[k], wt[k]) pair immediately feeds
                    # m_tiles matmuls (PE starts as soon as the first pair
                    # lands). Then finish per-m (k-inner) so the 8 psum banks
                    # complete staggered and evictions overlap compute.
                    k_half = min(3 * k_tiles // 4, kbf)
                    pss = [
                        ppool.tile([128, n_tile], mybir.dt.float32, name=f"ps{m}", tag="ps")
                        for m in range(m_tiles)
                    ]
                    for k in range(k_half):
                        for m in range(m_tiles):
                            nc.tensor.matmul(
                                pss[m][:],
                                xsl(k, m),
                                wsl(k),
                                start=(k == 0),
                                stop=False,
                                skip_group_check=True,
                            )
                    for m in range(m_tiles):
                        for k in range(k_half, kbf):
                            nc.tensor.matmul(
                                pss[m][:],
                                xsl(k, m),
                                wsl(k),
                                start=False,
                                stop=(kf8_pairs == 0 and k == kbf - 1),
                                skip_group_check=True,
                            )
                        if kf8_pairs:
                            dr_mms(pss[m][:], m, 0, n_tile, True)
                        evict(pss[m][:], m, 0, n_tile)
                else:
                    # m-outer / k-inner: staggered psum completion overlaps
                    # eviction + output DMA with compute. The very last group
                    # is split in half column-wise so the final eviction +
                    # output DMA (whose ~2us HBM write receipt is on the
                    # critical path) moves half as much data after the last
                    # matmul.
                    for m in range(m_tiles):
                        last = n == n_tiles - 1 and m == m_tiles - 1
                        if not last:
                            ps = ppool.tile(
                                [128, n_tile], mybir.dt.float32, name="ps", tag="ps"
                            )
                            for k in range(kbf):
                                nc.tensor.matmul(
                                    ps[:],
                                    xsl(k, m),
                                    wsl(k),
                                    start=(k == 0),
                                    stop=(kf8_pairs == 0 and k == kbf - 1),
                                )
                            if kf8_pairs:
                                dr_mms(ps[:], m, 0, n_tile, False)
                            evict(ps[:], m, 0, n_tile)
                        else:
                            half = n_tile // 2
                            for h in range(2):
                                ps = ppool.tile(
                                    [128, half], mybir.dt.float32, name="ps", tag="ps"
                                )
                                for k in range(kbf):
                                    nc.tensor.matmul(
                                        ps[:],
                                        xsl(k, m),
                                        wsl(k)[:, h * half : (h + 1) * half],
                                        start=(k == 0),
                                        stop=(kf8_pairs == 0 and k == kbf - 1),
                                    )
                                if kf8_pairs:
                                    dr_mms(ps[:], m, h * half, half, False)
                                evict(ps[:], m, h * half, half)
                if n + 1 < n_tiles:
                    wts = nxt
                    w8ts = nxt8
    nc.compile()
    return nc


def _get_nc(mode):
    if mode not in _NC_CACHE:
        _NC_CACHE[mode] = _build_nc(mode)
    return _NC_CACHE[mode]


def _soft_topk_scale(alpha):
    a = alpha.astype(np.float64)
    e = np.exp(a - a.max())
    return np.clip(KTOPK * (e / e.sum()), 0.0, 1.0).astype(np.float32)


def kernel(x, V, alpha):
    global _LAST_RESULTS
    from concourse.bass_utils import run_bass_kernel_spmd

    x = np.asarray(x, dtype=np.float32)
    V = np.asarray(V, dtype=np.float32)
    alpha = np.asarray(alpha, dtype=np.float32)

    a_topk = _soft_topk_scale(alpha)
    VsT = np.ascontiguousarray((V * a_topk[:, None]).T)  # [c, p]
    xT = np.ascontiguousarray(x.T)  # [F, B]

    mode = _MODE
    bf = ml_dtypes.bfloat16
    if mode == "mixed":
        kbf_rows = (F // 128 - 2 * KF8_PAIRS) * 128
        ws = np.float32(2.0**WSCALE_LOG2)
        VsT_s = VsT * ws
        W2 = np.concatenate(
            [VsT_s[:kbf_rows], VsT_s[:kbf_rows]], axis=1
        ).astype(bf)  # [kbf*128, 2F]
        W28 = np.concatenate(
            [VsT_s[kbf_rows:], VsT_s[kbf_rows:]], axis=1
        ).astype(ml_dtypes.float8_e4m3)  # [pairs*256, 2F]
        xTb = xT[:kbf_rows].astype(bf)
        xT8 = xT[kbf_rows:].astype(ml_dtypes.float8_e4m3)
    else:
        W2 = np.concatenate([VsT, VsT], axis=1).astype(bf)
        xTb = xT.astype(bf)

    nc = _get_nc(mode)
    in_maps = []
    for i in range(NCORES):
        m = {
            "xt": np.ascontiguousarray(xTb[:, i * BS : (i + 1) * BS]),
            "w2": W2,
        }
        if mode == "mixed":
            m["xt8"] = np.ascontiguousarray(xT8[:, i * BS : (i + 1) * BS])
            m["w28"] = W28
        in_maps.append(m)
    kwargs = {}
    if os.environ.get("GTOPK_TRACE"):
        try:
            import antenv.axon_hooks  # noqa: F401  (trace needs the hook)

            kwargs["trace"] = True
        except ImportError:
            pass
    res = run_bass_kernel_spmd(nc, in_maps, core_ids=list(range(NCORES)), **kwargs)
    _LAST_RESULTS = res
    return np.concatenate([r["out"] for r in res.results], axis=0)
